# revision 9
# baseline (speedup 1.0000x reference)
"""Bilinear CNN pooling kernel for Trainium2 (8 NeuronCores, data-parallel).

Computes, for each batch b:
    dotted[c,d] = sum_x left[b,x,c] * right[b,x,d]      (X = 112*112 = 12544)
    sqrted      = sign(dotted) * sqrt(|dotted| + 1e-9)
    out[b]      = sqrted / sqrt(sum(sqrted^2))          (flattened to [C*C])

Sharding: batch dim (32) split 4-per-core across 8 cores; no communication.
Note sum(sqrted^2) == sum(|dotted|) + C*C*eps exactly, so the L2 norm needs
only an abs-sum reduction, not a square pass.
"""

import sys

sys.path.insert(0, "/opt/trn_rl_repo")

import numpy as np

# ---- problem constants (hardcoded; kernel.py must be self-contained) ----
B = 32          # full batch
N_CORES = 8
BPC = B // N_CORES  # batches per core = 4
H = 112
W = 112
X = H * W       # 12544 contraction length
C = 128         # channels
P = 128         # partitions
NBLK = X // P   # 98 x-blocks of 128 rows

EPS_SQRT = 1e-9

# ---- tunables ----
MM_DTYPE = "bf16"    # "f32" (exact) or "bf16" (DMA-cast, ~1e-3 rel err, faster PE)
CHUNK_BLOCKS = 14    # x-blocks per DMA chunk (must divide 98): 2,7,14,49
BUFS = 4             # double/triple buffering depth for input tiles

_CACHE = {}


def _build_bass():
    import concourse.bass as bass
    import concourse.tile as tile
    from concourse import bacc
    from concourse import mybir
    from concourse import bass_isa
    from contextlib import ExitStack

    f32 = mybir.dt.float32
    mm_dt = f32 if MM_DTYPE == "f32" else mybir.dt.bfloat16
    AF = mybir.ActivationFunctionType

    nchunks = NBLK // CHUNK_BLOCKS
    assert nchunks * CHUNK_BLOCKS == NBLK

    nc = bacc.Bacc(None)
    left = nc.declare_dram_parameter("left", [BPC, X, C], f32, isOutput=False)
    right = nc.declare_dram_parameter("right", [BPC, X, C], f32, isOutput=False)
    out = nc.declare_dram_parameter("out", [BPC, C * C], f32, isOutput=True)

    with ExitStack() as ctx:
        tc = ctx.enter_context(tile.TileContext(nc))
        lpool = ctx.enter_context(tc.tile_pool(name="lpool", bufs=BUFS))
        rpool = ctx.enter_context(tc.tile_pool(name="rpool", bufs=BUFS))
        ppool = ctx.enter_context(tc.tile_pool(name="ppool", bufs=2, space="PSUM"))
        epool = ctx.enter_context(tc.tile_pool(name="epool", bufs=2))
        singles = ctx.enter_context(tc.tile_pool(name="singles", bufs=1))

        eps_tile = singles.tile([P, 1], f32)
        nc.vector.memset(eps_tile, EPS_SQRT)
        epsn_tile = singles.tile([P, 1], f32)
        nc.vector.memset(epsn_tile, float(C * C * EPS_SQRT))

        for b in range(BPC):
            # view [X, C] as [p=128, n=98, c=128]: partition = x % 128
            lv = left[b].rearrange("(n p) c -> p n c", p=P)
            rv = right[b].rearrange("(n p) c -> p n c", p=P)

            ps = ppool.tile([P, C], f32, tag="acc")
            for ci in range(nchunks):
                lt = lpool.tile([P, CHUNK_BLOCKS, C], mm_dt, tag="lt")
                rt = rpool.tile([P, CHUNK_BLOCKS, C], mm_dt, tag="rt")
                sl = slice(ci * CHUNK_BLOCKS, (ci + 1) * CHUNK_BLOCKS)
                if MM_DTYPE == "f32":
                    nc.sync.dma_start(out=lt, in_=lv[:, sl, :])
                    nc.sync.dma_start(out=rt, in_=rv[:, sl, :])
                else:
                    # SWDGE casts f32 -> bf16 inline during the DMA
                    nc.gpsimd.dma_start(out=lt, in_=lv[:, sl, :])
                    nc.gpsimd.dma_start(out=rt, in_=rv[:, sl, :])
                for i in range(CHUNK_BLOCKS):
                    g = ci * CHUNK_BLOCKS + i
                    nc.tensor.matmul(
                        ps,
                        lt[:, i, :],
                        rt[:, i, :],
                        start=(g == 0),
                        stop=(g == NBLK - 1),
                    )

            # ---- epilogue ----
            # sumsq = sum(|dotted|) over all C*C elements (+ C*C*eps const)
            asum = epool.tile([P, 1], f32, tag="asum")
            nc.vector.tensor_reduce(
                out=asum,
                in_=ps,
                axis=mybir.AxisListType.X,
                op=mybir.AluOpType.add,
                apply_absolute_value=True,
            )
            tot = epool.tile([P, 1], f32, tag="tot")
            nc.gpsimd.partition_all_reduce(
                tot, asum, channels=P, reduce_op=bass_isa.ReduceOp.add
            )
            # rb = 1 / sqrt(sumsq + C*C*eps)
            rb = epool.tile([P, 1], f32, tag="rb")
            nc.scalar.activation(rb, tot, AF.Sqrt, bias=epsn_tile)
            nc.vector.reciprocal(rb, rb)

            # sqrted = sign(dotted) * sqrt(|dotted| + eps)
            sg = epool.tile([P, C], f32, tag="sg")
            nc.scalar.activation(sg, ps, AF.Sign)
            av = epool.tile([P, C], f32, tag="av")
            nc.scalar.activation(av, ps, AF.Abs)
            tq = epool.tile([P, C], f32, tag="tq")
            nc.scalar.activation(tq, av, AF.Sqrt, bias=eps_tile)
            sq = epool.tile([P, C], f32, tag="sq")
            nc.vector.tensor_mul(sq, sg, tq)

            # normed = sqrted * rb
            normed = epool.tile([P, C], f32, tag="normed")
            nc.vector.tensor_scalar_mul(normed, sq, rb)

            nc.sync.dma_start(out=out[b].rearrange("(c d) -> c d", d=C), in_=normed)

    nc.finalize()
    return nc


def _get_nc():
    key = (MM_DTYPE, CHUNK_BLOCKS, BUFS)
    if key not in _CACHE:
        _CACHE[key] = _build_bass()
    return _CACHE[key]


def run(left, right, trace=False, **kw):
    """Shard inputs, run the SPMD bass kernel on 8 cores, gather outputs.

    Returns (output [32, 16384] f32, BassKernelResults)."""
    from concourse import bass_utils

    left = np.ascontiguousarray(left, dtype=np.float32).reshape(B, X, C)
    right = np.ascontiguousarray(right, dtype=np.float32).reshape(B, X, C)

    nc = _get_nc()
    in_maps = []
    for i in range(N_CORES):
        sl = slice(i * BPC, (i + 1) * BPC)
        in_maps.append({"left": left[sl], "right": right[sl]})

    res = bass_utils.run_bass_kernel_spmd(
        nc, in_maps, core_ids=list(range(N_CORES)), trace=trace, **kw
    )
    outs = np.concatenate([res.results[i]["out"] for i in range(N_CORES)], axis=0)
    return outs, res


def kernel(**inputs):
    out, _ = run(inputs["left"], inputs["right"])
    return out
